# revision 21
# baseline (speedup 1.0000x reference)
"""Trainium2 Bass kernel for ChannelSpatialSELayer (cSE + sSE squeeze-excite).

    out = max(x * sigmoid(MLP(mean_dhw(x))),          # channel gate (per b, c)
              x * sigmoid(conv_w . x + conv_b))       # spatial gate (per b,d,h,w)

Sharding: pure data parallel over the 64 (batch, depth) slices -> 8 slices
per core.  Cores 0-3 hold batch 0, cores 4-7 hold batch 1.  The only
cross-core dependency is the channel mean, whose per-core partial sums
(128 floats) are AllReduced within each batch's 4-core replica group.

Per core, x stays resident in SBUF between the stats pass and the apply
pass, so HBM traffic is one read + one write of the shard:
  pass 1: DMA x -> SBUF [128, 4*9216] (2 slices x 64 chans on partitions),
          DVE per-channel partial sums, PE matmul sq = conv_w . x
          (channel-selector lhsT), ACT sigmoid PSUM -> sq16 SBUF.
  ...   : AllReduce(128 floats), tiny MLP on PE/ACT -> per-partition gate.
  pass 2: PE broadcast-matmul spatial gate to 128 partitions,
          DVE t2 = x*gs, DVE out = (x*gc) max t2, DMA out.
"""

import numpy as np

import concourse.bass as bass
import concourse.mybir as mybir
import concourse.tile as tile
from concourse import bacc
from concourse.bass_utils import run_bass_kernel_spmd

B, C, D, H, W = 2, 64, 32, 96, 96
CR = C // 2
S = H * W                 # 9216 spatial elements per (b, d) slice
NCORES = 8
SL = 8                    # (b, d) slices per core
NPAIR = SL // 2           # 4 resident [128, S] slabs per core
NMEAN = float(D * H * W)  # divisor of the channel mean

LOAD = 2304               # pass-1 load chunk (columns)
MCH = 1536                # sq PSUM chunk = 3 banks
PCH = 1536                # pass-2 chunk = 3 banks
GROUPS = [[0, 1, 2, 3], [4, 5, 6, 7]]  # batch replica groups

F32 = mybir.dt.float32
AX = mybir.AxisListType
AL = mybir.AluOpType
AF = mybir.ActivationFunctionType


def _build(fc1_w, fc1_b, fc2_w, fc2_b, conv_w, conv_b):
    # Bacc (not raw Bass): its compile() pipeline splits multi-sem waits
    # into event semaphores — TRN2 allows at most 1 wait per instruction.
    nc = bacc.Bacc(
        "TRN2",
        target_bir_lowering=False,
        debug=False,
        num_devices=NCORES,
    )
    xin = nc.dram_tensor("xin", [C, SL, S], F32, kind="ExternalInput")
    yout = nc.dram_tensor("yout", [C, SL, S], F32, kind="ExternalOutput")

    # Host-prepared constants (identical on every core, embedded in the NEFF).
    # w1fold folds the 1/NMEAN of the mean into fc1 and sums the two
    # 64-partition halves (both hold the same batch) in the K=128 contraction.
    w1fold = (np.vstack([fc1_w.T, fc1_w.T]) / NMEAN).astype(np.float32)  # [128,CR]
    w2t = np.ascontiguousarray(fc2_w.T).astype(np.float32)               # [CR,C]
    wsel = np.zeros((128, 2), np.float32)  # sq = wsel.T @ x per slice pair
    wsel[:C, 0] = conv_w
    wsel[C:, 1] = conv_w
    # broadcast-selector: pair jp's two gs rows live at partition base
    # 32*jp (the only legal SBUF engine bases are 0/32/64/96).  lhsT
    # [2, 128] at that base sends row 0 to partitions 0-63 and row 1 to
    # partitions 64-127 of the PSUM output.
    bselbig = np.zeros((98, 128), np.float32)
    for jp in range(NPAIR):
        bselbig[32 * jp, :C] = 1.0
        bselbig[32 * jp + 1, C:] = 1.0
    dup = np.zeros((C, 128), np.float32)   # duplicate gc [64] -> [128]
    dup[np.arange(C), np.arange(C)] = 1.0
    dup[np.arange(C), C + np.arange(C)] = 1.0
    b1 = fc1_b.reshape(CR, 1).astype(np.float32)
    b2 = fc2_b.reshape(C, 1).astype(np.float32)
    cb = float(np.asarray(conv_b).reshape(-1)[0])

    w1_d = nc.inline_tensor(w1fold, "w1fold")
    w2_d = nc.inline_tensor(w2t, "w2t")
    wsel_d = nc.inline_tensor(wsel, "wsel")
    bsel_d = nc.inline_tensor(bselbig, "bselbig")
    dup_d = nc.inline_tensor(dup, "dup")
    b1_d = nc.inline_tensor(b1, "b1")
    b2_d = nc.inline_tensor(b2, "b2")

    with tile.TileContext(nc) as tc:
        with (
            tc.tile_pool(name="consts", bufs=1) as consts,
            tc.tile_pool(name="xpool", bufs=1) as xpool,
            tc.tile_pool(name="sqpool", bufs=1) as sqpool,
            tc.tile_pool(name="stp", bufs=1) as stp,
            tc.tile_pool(name="dram", bufs=1, space="DRAM") as dram,
        ):
            wsel_sb = consts.tile([128, 2], F32)
            nc.sync.dma_start(out=wsel_sb, in_=wsel_d[:, :])
            bsel_sb = consts.tile([98, 128], F32)
            nc.sync.dma_start(out=bsel_sb, in_=bsel_d[:, :])
            dup_sb = consts.tile([C, 128], F32)
            nc.sync.dma_start(out=dup_sb, in_=dup_d[:, :])
            w1_sb = consts.tile([128, CR], F32)
            nc.sync.dma_start(out=w1_sb, in_=w1_d[:, :])
            w2_sb = consts.tile([CR, C], F32)
            nc.sync.dma_start(out=w2_sb, in_=w2_d[:, :])
            b1_sb = consts.tile([CR, 1], F32)
            nc.sync.dma_start(out=b1_sb, in_=b1_d[:, :])
            b2_sb = consts.tile([C, 1], F32)
            nc.sync.dma_start(out=b2_sb, in_=b2_d[:, :])
            cbB = consts.tile([98, 1], F32)
            nc.vector.memset(cbB, cb)

            xres = xpool.tile([128, NPAIR * S], F32)   # 144 KB/partition
            # spatial gates: pair jp's two rows sit at partition base 32*jp
            sqb = sqpool.tile([98, S], F32)
            stats = stp.tile([128, 16], F32)

            # ---------- pass 1: load resident x, channel sums, sq logits ----
            with tc.tile_pool(name="psq", bufs=2, space="PSUM") as psq:
                for jp in range(NPAIR):
                    xin_pair = xin[:, 2 * jp : 2 * jp + 2, :].rearrange(
                        "c t s -> t c s"
                    )
                    for lc in range(S // LOAD):
                        c0 = lc * LOAD
                        dst = xres[:, jp * S + c0 : jp * S + c0 + LOAD]
                        nc.sync.dma_start(
                            out=dst,
                            in_=xin_pair[:, :, c0 : c0 + LOAD],
                        )
                        nc.vector.reduce_sum(
                            out=stats[:, jp * 4 + lc : jp * 4 + lc + 1],
                            in_=dst,
                            axis=AX.X,
                        )
                    r0 = 32 * jp
                    for mc in range(S // MCH):
                        ps = psq.tile([128, MCH], F32)
                        for i in range(MCH // 512):
                            o = mc * MCH + i * 512
                            nc.tensor.matmul(
                                ps[r0 : r0 + 2, i * 512 : (i + 1) * 512],
                                lhsT=wsel_sb,
                                rhs=xres[:, jp * S + o : jp * S + o + 512],
                                start=True,
                                stop=True,
                                tile_position=(0, r0),
                            )
                        off = mc * MCH
                        nc.scalar.activation(
                            out=sqb[r0 : r0 + 2, off : off + MCH],
                            in_=ps[r0 : r0 + 2, :],
                            func=AF.Sigmoid,
                            bias=cbB[r0 : r0 + 2, :],
                            scale=1.0,
                        )

            # ---------- channel-sum AllReduce within the batch group --------
            ssum = stp.tile([128, 1], F32)
            nc.vector.reduce_sum(out=ssum, in_=stats, axis=AX.X)
            b_in = dram.tile([128, 1], F32)
            b_out = dram.tile([128, 1], F32)
            nc.sync.dma_start(out=b_in, in_=ssum)
            nc.gpsimd.collective_compute(
                "AllReduce",
                AL.add,
                replica_groups=GROUPS,
                ins=[b_in.opt()],
                outs=[b_out.opt()],
            )
            s_sb = stp.tile([128, 1], F32)
            nc.sync.dma_start(out=s_sb, in_=b_out)

            # ---------- tiny cSE MLP -> per-partition channel gate ----------
            with tc.tile_pool(name="pmlp", bufs=1, space="PSUM") as pmlp:
                h_ps = pmlp.tile([CR, 1], F32)
                nc.tensor.matmul(h_ps, lhsT=w1_sb, rhs=s_sb, start=True, stop=True)
                h_sb = stp.tile([CR, 1], F32)
                nc.scalar.activation(
                    out=h_sb, in_=h_ps, func=AF.Relu, bias=b1_sb, scale=1.0
                )
                g_ps = pmlp.tile([C, 1], F32)
                nc.tensor.matmul(g_ps, lhsT=w2_sb, rhs=h_sb, start=True, stop=True)
                gc_sb = stp.tile([C, 1], F32)
                nc.scalar.activation(
                    out=gc_sb, in_=g_ps, func=AF.Sigmoid, bias=b2_sb, scale=1.0
                )
                g2_ps = pmlp.tile([128, 1], F32)
                nc.tensor.matmul(g2_ps, lhsT=dup_sb, rhs=gc_sb, start=True, stop=True)
                g2_sb = stp.tile([128, 1], F32)
                nc.vector.tensor_copy(out=g2_sb, in_=g2_ps)

            # ---------- pass 2: apply both gates, stream out ----------------
            with (
                tc.tile_pool(name="pb", bufs=2, space="PSUM") as pb,
                tc.tile_pool(name="t2p", bufs=3) as t2p,
            ):
                for jp in range(NPAIR):
                    yv = yout[:, 2 * jp : 2 * jp + 2, :].rearrange("c t s -> t c s")
                    r0 = 32 * jp
                    for pc in range(S // PCH):
                        o = pc * PCH
                        xc = xres[:, jp * S + o : jp * S + o + PCH]
                        g_ps = pb.tile([128, PCH], F32)
                        for i in range(PCH // 512):
                            nc.tensor.matmul(
                                g_ps[:, i * 512 : (i + 1) * 512],
                                lhsT=bsel_sb[r0 : r0 + 2, :],
                                rhs=sqb[r0 : r0 + 2, o + i * 512 : o + (i + 1) * 512],
                                start=True,
                                stop=True,
                                tile_position=(r0, 0),
                            )
                        t2 = t2p.tile([128, PCH], F32)
                        nc.vector.tensor_mul(out=t2, in0=xc, in1=g_ps)
                        nc.vector.scalar_tensor_tensor(
                            out=t2,
                            in0=xc,
                            scalar=g2_sb,
                            in1=t2,
                            op0=AL.mult,
                            op1=AL.max,
                        )
                        nc.sync.dma_start(
                            out=yv[:, :, o : o + PCH],
                            in_=t2,
                        )
    # run Bacc's compile pipeline (register allocation, wait splitting);
    # the bass2jax/PJRT runner does not finalize on its own.
    nc.finalize()
    return nc


def _shard(x):
    in_maps = []
    for k in range(NCORES):
        b, d0 = k // 4, SL * (k % 4)
        shard = np.ascontiguousarray(x[b, :, d0 : d0 + SL].reshape(C, SL, S))
        in_maps.append({"xin": shard})
    return in_maps


def _unshard(results):
    out = np.empty((B, C, D, H, W), np.float32)
    for k in range(NCORES):
        b, d0 = k // 4, SL * (k % 4)
        out[b, :, d0 : d0 + SL] = results[k]["yout"].reshape(C, SL, H, W)
    return out


def _run(inputs, trace=False):
    x = np.ascontiguousarray(np.asarray(inputs["input_tensor"], dtype=np.float32))
    ws = [
        np.asarray(inputs[k], dtype=np.float32)
        for k in ("fc1_w", "fc1_b", "fc2_w", "fc2_b", "conv_w", "conv_b")
    ]
    nc = _build(*ws)
    res = run_bass_kernel_spmd(nc, _shard(x), list(range(NCORES)), trace=trace)
    return _unshard(res.results), res


def kernel(**inputs):
    out, _ = _run(inputs, trace=False)
    return out


# revision 26
# speedup vs baseline: 2.8982x; 2.8982x over previous
"""Trainium2 Bass kernel for ChannelSpatialSELayer (cSE + sSE squeeze-excite).

    out = max(x * sigmoid(MLP(mean_dhw(x))),          # channel gate (per b, c)
              x * sigmoid(conv_w . x + conv_b))       # spatial gate (per b,d,h,w)

Sharding: pure data parallel over the 64 (batch, depth) slices -> 8 slices
per core.  Cores 0-3 hold batch 0, cores 4-7 hold batch 1.  The only
cross-core dependency is the channel mean, whose per-core partial sums
(128 floats) are AllReduced within each batch's 4-core replica group.

Per core, x stays resident in SBUF between the stats pass and the apply
pass, so HBM traffic is one read + one write of the shard:
  pass 1: DMA x -> SBUF [128, 4*9216] (2 slices x 64 chans on partitions),
          DVE per-channel partial sums, PE matmul sq = conv_w . x
          (channel-selector lhsT), ACT sigmoid PSUM -> sq16 SBUF.
  ...   : AllReduce(128 floats), tiny MLP on PE/ACT -> per-partition gate.
  pass 2: PE broadcast-matmul spatial gate to 128 partitions,
          DVE t2 = x*gs, DVE out = (x*gc) max t2, DMA out.
"""

import numpy as np

import concourse.bass as bass
import concourse.mybir as mybir
import concourse.tile as tile
from concourse import bacc
from concourse.bass_utils import run_bass_kernel_spmd

B, C, D, H, W = 2, 64, 32, 96, 96
CR = C // 2
S = H * W                 # 9216 spatial elements per (b, d) slice
NCORES = 8
SL = 8                    # (b, d) slices per core
NPAIR = SL // 2           # 4 resident [128, S] slabs per core
NMEAN = float(D * H * W)  # divisor of the channel mean

LOAD = 2304               # pass-1 load chunk (columns)
MCH = 1536                # sq PSUM chunk = 3 banks
PCH = 1536                # pass-2 chunk = 3 banks
GROUPS = [[0, 1, 2, 3], [4, 5, 6, 7]]  # batch replica groups

F32 = mybir.dt.float32
AX = mybir.AxisListType
AL = mybir.AluOpType
AF = mybir.ActivationFunctionType


def _build(fc1_w, fc1_b, fc2_w, fc2_b, conv_w, conv_b):
    # Bacc (not raw Bass): its compile() pipeline splits multi-sem waits
    # into event semaphores — TRN2 allows at most 1 wait per instruction.
    nc = bacc.Bacc(
        "TRN2",
        target_bir_lowering=False,
        debug=False,
        num_devices=NCORES,
    )
    # [pair, partition, spatial]: the host pre-arranges shards so every DMA's
    # outer dim is the full 128 partitions — the SDMA engine fan-out follows
    # the outer AP dim in groups of 8, so this engages all 16 engines.
    xin = nc.dram_tensor("xin", [NPAIR, 128, S], F32, kind="ExternalInput")
    yout = nc.dram_tensor("yout", [NPAIR, 128, S], F32, kind="ExternalOutput")

    # Host-prepared constants (identical on every core, embedded in the NEFF).
    # w1fold folds the 1/NMEAN of the mean into fc1 and sums the two
    # 64-partition halves (both hold the same batch) in the K=128 contraction.
    w1fold = (np.vstack([fc1_w.T, fc1_w.T]) / NMEAN).astype(np.float32)  # [128,CR]
    w2t = np.ascontiguousarray(fc2_w.T).astype(np.float32)               # [CR,C]
    wsel = np.zeros((128, 2), np.float32)  # sq = wsel.T @ x per slice pair
    wsel[:C, 0] = conv_w
    wsel[C:, 1] = conv_w
    # broadcast-selector: pair jp's two gs rows live at partition base
    # 32*jp (the only legal SBUF engine bases are 0/32/64/96).  lhsT
    # [2, 128] at that base sends row 0 to partitions 0-63 and row 1 to
    # partitions 64-127 of the PSUM output.
    bselbig = np.zeros((98, 128), np.float32)
    for jp in range(NPAIR):
        bselbig[32 * jp, :C] = 1.0
        bselbig[32 * jp + 1, C:] = 1.0
    dup = np.zeros((C, 128), np.float32)   # duplicate gc [64] -> [128]
    dup[np.arange(C), np.arange(C)] = 1.0
    dup[np.arange(C), C + np.arange(C)] = 1.0
    b1 = fc1_b.reshape(CR, 1).astype(np.float32)
    b2 = fc2_b.reshape(C, 1).astype(np.float32)
    cb = float(np.asarray(conv_b).reshape(-1)[0])

    w1_d = nc.inline_tensor(w1fold, "w1fold")
    w2_d = nc.inline_tensor(w2t, "w2t")
    wsel_d = nc.inline_tensor(wsel, "wsel")
    bsel_d = nc.inline_tensor(bselbig, "bselbig")
    dup_d = nc.inline_tensor(dup, "dup")
    b1_d = nc.inline_tensor(b1, "b1")
    b2_d = nc.inline_tensor(b2, "b2")

    with tile.TileContext(nc) as tc:
        with (
            tc.tile_pool(name="consts", bufs=1) as consts,
            tc.tile_pool(name="xpool", bufs=1) as xpool,
            tc.tile_pool(name="sqpool", bufs=1) as sqpool,
            tc.tile_pool(name="stp", bufs=1) as stp,
            tc.tile_pool(name="dram", bufs=1, space="DRAM") as dram,
        ):
            wsel_sb = consts.tile([128, 2], F32)
            nc.sync.dma_start(out=wsel_sb, in_=wsel_d[:, :])
            bsel_sb = consts.tile([98, 128], F32)
            nc.sync.dma_start(out=bsel_sb, in_=bsel_d[:, :])
            dup_sb = consts.tile([C, 128], F32)
            nc.sync.dma_start(out=dup_sb, in_=dup_d[:, :])
            w1_sb = consts.tile([128, CR], F32)
            nc.sync.dma_start(out=w1_sb, in_=w1_d[:, :])
            w2_sb = consts.tile([CR, C], F32)
            nc.sync.dma_start(out=w2_sb, in_=w2_d[:, :])
            b1_sb = consts.tile([CR, 1], F32)
            nc.sync.dma_start(out=b1_sb, in_=b1_d[:, :])
            b2_sb = consts.tile([C, 1], F32)
            nc.sync.dma_start(out=b2_sb, in_=b2_d[:, :])
            cbB = consts.tile([98, 1], F32)
            nc.vector.memset(cbB, cb)

            xres = xpool.tile([128, NPAIR * S], F32)   # 144 KB/partition
            # spatial gates: pair jp's two rows sit at partition base 32*jp
            sqb = sqpool.tile([98, S], F32)
            stats = stp.tile([128, 16], F32)

            # ---------- pass 1: load resident x, channel sums, sq logits ----
            with tc.tile_pool(name="psq", bufs=2, space="PSUM") as psq:
                for jp in range(NPAIR):
                    for lc in range(S // LOAD):
                        c0 = lc * LOAD
                        dst = xres[:, jp * S + c0 : jp * S + c0 + LOAD]
                        nc.sync.dma_start(
                            out=dst,
                            in_=xin[jp, :, c0 : c0 + LOAD],
                        )
                        nc.vector.reduce_sum(
                            out=stats[:, jp * 4 + lc : jp * 4 + lc + 1],
                            in_=dst,
                            axis=AX.X,
                        )
                    r0 = 32 * jp
                    for mc in range(S // MCH):
                        ps = psq.tile([128, MCH], F32)
                        for i in range(MCH // 512):
                            o = mc * MCH + i * 512
                            nc.tensor.matmul(
                                ps[r0 : r0 + 2, i * 512 : (i + 1) * 512],
                                lhsT=wsel_sb,
                                rhs=xres[:, jp * S + o : jp * S + o + 512],
                                start=True,
                                stop=True,
                                tile_position=(0, r0),
                            )
                        off = mc * MCH
                        nc.scalar.activation(
                            out=sqb[r0 : r0 + 2, off : off + MCH],
                            in_=ps[r0 : r0 + 2, :],
                            func=AF.Sigmoid,
                            bias=cbB[r0 : r0 + 2, :],
                            scale=1.0,
                        )

            # ---------- channel-sum AllReduce within the batch group --------
            ssum = stp.tile([128, 1], F32)
            nc.vector.reduce_sum(out=ssum, in_=stats, axis=AX.X)
            b_in = dram.tile([128, 1], F32)
            b_out = dram.tile([128, 1], F32)
            nc.sync.dma_start(out=b_in, in_=ssum)
            nc.gpsimd.collective_compute(
                "AllReduce",
                AL.add,
                replica_groups=GROUPS,
                ins=[b_in.opt()],
                outs=[b_out.opt()],
            )
            s_sb = stp.tile([128, 1], F32)
            nc.sync.dma_start(out=s_sb, in_=b_out)

            # ---------- tiny cSE MLP -> per-partition channel gate ----------
            with tc.tile_pool(name="pmlp", bufs=1, space="PSUM") as pmlp:
                h_ps = pmlp.tile([CR, 1], F32)
                nc.tensor.matmul(h_ps, lhsT=w1_sb, rhs=s_sb, start=True, stop=True)
                h_sb = stp.tile([CR, 1], F32)
                nc.scalar.activation(
                    out=h_sb, in_=h_ps, func=AF.Relu, bias=b1_sb, scale=1.0
                )
                g_ps = pmlp.tile([C, 1], F32)
                nc.tensor.matmul(g_ps, lhsT=w2_sb, rhs=h_sb, start=True, stop=True)
                gc_sb = stp.tile([C, 1], F32)
                nc.scalar.activation(
                    out=gc_sb, in_=g_ps, func=AF.Sigmoid, bias=b2_sb, scale=1.0
                )
                g2_ps = pmlp.tile([128, 1], F32)
                nc.tensor.matmul(g2_ps, lhsT=dup_sb, rhs=gc_sb, start=True, stop=True)
                g2_sb = stp.tile([128, 1], F32)
                nc.vector.tensor_copy(out=g2_sb, in_=g2_ps)

            # ---------- pass 2: apply both gates, stream out ----------------
            with (
                tc.tile_pool(name="pb", bufs=2, space="PSUM") as pb,
                tc.tile_pool(name="t2p", bufs=3) as t2p,
            ):
                for jp in range(NPAIR):
                    r0 = 32 * jp
                    for pc in range(S // PCH):
                        o = pc * PCH
                        xc = xres[:, jp * S + o : jp * S + o + PCH]
                        g_ps = pb.tile([128, PCH], F32)
                        for i in range(PCH // 512):
                            nc.tensor.matmul(
                                g_ps[:, i * 512 : (i + 1) * 512],
                                lhsT=bsel_sb[r0 : r0 + 2, :],
                                rhs=sqb[r0 : r0 + 2, o + i * 512 : o + (i + 1) * 512],
                                start=True,
                                stop=True,
                                tile_position=(r0, 0),
                            )
                        t2 = t2p.tile([128, PCH], F32)
                        nc.vector.tensor_mul(out=t2, in0=xc, in1=g_ps)
                        nc.vector.scalar_tensor_tensor(
                            out=t2,
                            in0=xc,
                            scalar=g2_sb,
                            in1=t2,
                            op0=AL.mult,
                            op1=AL.max,
                        )
                        nc.sync.dma_start(
                            out=yout[jp, :, o : o + PCH],
                            in_=t2,
                        )
    # run Bacc's compile pipeline (register allocation, wait splitting);
    # the bass2jax/PJRT runner does not finalize on its own.
    nc.finalize()
    return nc


def _shard(x):
    # core k shard: xin[jp, 64*t + c, s] = x[b, c, d0 + 2*jp + t, s]
    in_maps = []
    for k in range(NCORES):
        b, d0 = k // 4, SL * (k % 4)
        v = x[b, :, d0 : d0 + SL].reshape(C, NPAIR, 2, S)
        shard = np.ascontiguousarray(v.transpose(1, 2, 0, 3).reshape(NPAIR, 128, S))
        in_maps.append({"xin": shard})
    return in_maps


def _unshard(results):
    out = np.empty((B, C, D, H, W), np.float32)
    for k in range(NCORES):
        b, d0 = k // 4, SL * (k % 4)
        y = results[k]["yout"].reshape(NPAIR, 2, C, S)
        out[b, :, d0 : d0 + SL] = y.transpose(2, 0, 1, 3).reshape(C, SL, H, W)
    return out


def _run(inputs, trace=False):
    x = np.ascontiguousarray(np.asarray(inputs["input_tensor"], dtype=np.float32))
    ws = [
        np.asarray(inputs[k], dtype=np.float32)
        for k in ("fc1_w", "fc1_b", "fc2_w", "fc2_b", "conv_w", "conv_b")
    ]
    nc = _build(*ws)
    res = run_bass_kernel_spmd(nc, _shard(x), list(range(NCORES)), trace=trace)
    return _unshard(res.results), res


def kernel(**inputs):
    out, _ = _run(inputs, trace=False)
    return out


# revision 30
# speedup vs baseline: 3.4849x; 1.2024x over previous
"""Trainium2 Bass kernel for ChannelSpatialSELayer (cSE + sSE squeeze-excite).

    out = max(x * sigmoid(MLP(mean_dhw(x))),          # channel gate (per b, c)
              x * sigmoid(conv_w . x + conv_b))       # spatial gate (per b,d,h,w)

Sharding: pure data parallel over the 64 (batch, depth) slices -> 8 slices
per core.  Cores 0-3 hold batch 0, cores 4-7 hold batch 1.  The only
cross-core dependency is the channel mean, whose per-core partial sums
(128 floats) are AllReduced within each batch's 4-core replica group.

Per core, x stays resident in SBUF between the stats pass and the apply
pass, so HBM traffic is one read + one write of the shard:
  pass 1: DMA x -> SBUF [128, 4*9216] (2 slices x 64 chans on partitions),
          DVE per-channel partial sums, PE matmul sq = conv_w . x
          (channel-selector lhsT), ACT sigmoid PSUM -> sq16 SBUF.
  ...   : AllReduce(128 floats), tiny MLP on PE/ACT -> per-partition gate.
  pass 2: PE broadcast-matmul spatial gate to 128 partitions,
          DVE t2 = x*gs, DVE out = (x*gc) max t2, DMA out.
"""

import numpy as np

import concourse.bass as bass
import concourse.mybir as mybir
import concourse.tile as tile
from concourse import bacc
from concourse.bass_utils import run_bass_kernel_spmd

B, C, D, H, W = 2, 64, 32, 96, 96
CR = C // 2
S = H * W                 # 9216 spatial elements per (b, d) slice
NCORES = 8
SL = 8                    # (b, d) slices per core
NPAIR = SL // 2           # 4 resident [128, S] slabs per core
NMEAN = float(D * H * W)  # divisor of the channel mean

LOAD = 2304               # pass-1 load chunk (columns)
MCH = 1024                # sq PSUM chunk = 2 banks
PCH = 1024                # pass-2 chunk = 2 banks
GROUPS = [[0, 1, 2, 3], [4, 5, 6, 7]]  # batch replica groups

F32 = mybir.dt.float32
AX = mybir.AxisListType
AL = mybir.AluOpType
AF = mybir.ActivationFunctionType


def _build(fc1_w, fc1_b, fc2_w, fc2_b, conv_w, conv_b):
    # Bacc (not raw Bass): its compile() pipeline splits multi-sem waits
    # into event semaphores — TRN2 allows at most 1 wait per instruction.
    nc = bacc.Bacc(
        "TRN2",
        target_bir_lowering=False,
        debug=False,
        num_devices=NCORES,
    )
    # [pair, partition, spatial]: the host pre-arranges shards so every DMA's
    # outer dim is the full 128 partitions — the SDMA engine fan-out follows
    # the outer AP dim in groups of 8, so this engages all 16 engines.
    xin = nc.dram_tensor("xin", [NPAIR, 128, S], F32, kind="ExternalInput")
    yout = nc.dram_tensor("yout", [NPAIR, 128, S], F32, kind="ExternalOutput")

    # Host-prepared constants (identical on every core, embedded in the NEFF).
    # w1fold folds the 1/NMEAN of the mean into fc1 and sums the two
    # 64-partition halves (both hold the same batch) in the K=128 contraction.
    w1fold = (np.vstack([fc1_w.T, fc1_w.T]) / NMEAN).astype(np.float32)  # [128,CR]
    w2t = np.ascontiguousarray(fc2_w.T).astype(np.float32)               # [CR,C]
    wsel = np.zeros((128, 2), np.float32)  # sq = wsel.T @ x per slice pair
    wsel[:C, 0] = conv_w
    wsel[C:, 1] = conv_w
    # broadcast-selector: pair jp's two gs rows live at partition base
    # 32*jp (the only legal SBUF engine bases are 0/32/64/96).  lhsT
    # [2, 128] at that base sends row 0 to partitions 0-63 and row 1 to
    # partitions 64-127 of the PSUM output.
    bselbig = np.zeros((98, 128), np.float32)
    for jp in range(NPAIR):
        bselbig[32 * jp, :C] = 1.0
        bselbig[32 * jp + 1, C:] = 1.0
    dup = np.zeros((C, 128), np.float32)   # duplicate gc [64] -> [128]
    dup[np.arange(C), np.arange(C)] = 1.0
    dup[np.arange(C), C + np.arange(C)] = 1.0
    b1 = fc1_b.reshape(CR, 1).astype(np.float32)
    b2 = fc2_b.reshape(C, 1).astype(np.float32)
    cb = float(np.asarray(conv_b).reshape(-1)[0])

    w1_d = nc.inline_tensor(w1fold, "w1fold")
    w2_d = nc.inline_tensor(w2t, "w2t")
    wsel_d = nc.inline_tensor(wsel, "wsel")
    bsel_d = nc.inline_tensor(bselbig, "bselbig")
    dup_d = nc.inline_tensor(dup, "dup")
    b1_d = nc.inline_tensor(b1, "b1")
    b2_d = nc.inline_tensor(b2, "b2")

    with tile.TileContext(nc) as tc:
        with (
            tc.tile_pool(name="consts", bufs=1) as consts,
            tc.tile_pool(name="xpool", bufs=1) as xpool,
            tc.tile_pool(name="sqpool", bufs=1) as sqpool,
            tc.tile_pool(name="stp", bufs=1) as stp,
            tc.tile_pool(name="dram", bufs=1, space="DRAM") as dram,
        ):
            wsel_sb = consts.tile([128, 2], F32)
            nc.sync.dma_start(out=wsel_sb, in_=wsel_d[:, :])
            bsel_sb = consts.tile([98, 128], F32)
            nc.sync.dma_start(out=bsel_sb, in_=bsel_d[:, :])
            dup_sb = consts.tile([C, 128], F32)
            nc.sync.dma_start(out=dup_sb, in_=dup_d[:, :])
            w1_sb = consts.tile([128, CR], F32)
            nc.sync.dma_start(out=w1_sb, in_=w1_d[:, :])
            w2_sb = consts.tile([CR, C], F32)
            nc.sync.dma_start(out=w2_sb, in_=w2_d[:, :])
            b1_sb = consts.tile([CR, 1], F32)
            nc.sync.dma_start(out=b1_sb, in_=b1_d[:, :])
            b2_sb = consts.tile([C, 1], F32)
            nc.sync.dma_start(out=b2_sb, in_=b2_d[:, :])
            cbB = consts.tile([98, 1], F32)
            nc.vector.memset(cbB, cb)

            xres = xpool.tile([128, NPAIR * S], F32)   # 144 KB/partition
            # spatial gates: pair jp's two rows sit at partition base 32*jp
            sqb = sqpool.tile([98, S], F32)
            stats = stp.tile([128, 16], F32)

            # ---------- pass 1: load resident x, channel sums, sq logits ----
            # psq (4 banks) and pb (4 banks) are open CONCURRENTLY so pass-2
            # broadcast matmuls need not wait for pass-1 PSUM releases.
            with (
                tc.tile_pool(name="psq", bufs=2, space="PSUM") as psq,
                tc.tile_pool(name="pb", bufs=2, space="PSUM") as pb,
                tc.tile_pool(name="t2p", bufs=3) as t2p,
            ):
                for jp in range(NPAIR):
                    for lc in range(S // LOAD):
                        c0 = lc * LOAD
                        dst = xres[:, jp * S + c0 : jp * S + c0 + LOAD]
                        nc.sync.dma_start(
                            out=dst,
                            in_=xin[jp, :, c0 : c0 + LOAD],
                        )
                        nc.vector.reduce_sum(
                            out=stats[:, jp * 4 + lc : jp * 4 + lc + 1],
                            in_=dst,
                            axis=AX.X,
                        )
                    r0 = 32 * jp
                    for mc in range(S // MCH):
                        ps = psq.tile([128, MCH], F32, tag="ps")
                        for i in range(MCH // 512):
                            o = mc * MCH + i * 512
                            nc.tensor.matmul(
                                ps[r0 : r0 + 2, i * 512 : (i + 1) * 512],
                                lhsT=wsel_sb,
                                rhs=xres[:, jp * S + o : jp * S + o + 512],
                                start=True,
                                stop=True,
                                tile_position=(0, r0),
                            )
                        off = mc * MCH
                        nc.scalar.activation(
                            out=sqb[r0 : r0 + 2, off : off + MCH],
                            in_=ps[r0 : r0 + 2, :],
                            func=AF.Sigmoid,
                            bias=cbB[r0 : r0 + 2, :],
                            scale=1.0,
                        )

                # ------- channel-sum AllReduce within the batch group -------
                ssum = stp.tile([128, 1], F32)
                nc.vector.reduce_sum(out=ssum, in_=stats, axis=AX.X)
                b_in = dram.tile([128, 1], F32)
                b_out = dram.tile([128, 1], F32)
                nc.sync.dma_start(out=b_in, in_=ssum)
                nc.gpsimd.collective_compute(
                    "AllReduce",
                    AL.add,
                    replica_groups=GROUPS,
                    ins=[b_in.opt()],
                    outs=[b_out.opt()],
                )
                s_sb = stp.tile([128, 1], F32)
                nc.sync.dma_start(out=s_sb, in_=b_out)

                # ------- tiny cSE MLP -> per-partition channel gate ---------
                # MLP PSUM lives in column 0 of psq-pool tiles (no extra banks)
                mt1 = psq.tile([128, MCH], F32, tag="ps")
                nc.tensor.matmul(
                    mt1[:CR, 0:1], lhsT=w1_sb, rhs=s_sb, start=True, stop=True
                )
                h_sb = stp.tile([CR, 1], F32)
                nc.scalar.activation(
                    out=h_sb, in_=mt1[:CR, 0:1], func=AF.Relu, bias=b1_sb, scale=1.0
                )
                mt2 = psq.tile([128, MCH], F32, tag="ps")
                nc.tensor.matmul(
                    mt2[:C, 0:1], lhsT=w2_sb, rhs=h_sb, start=True, stop=True
                )
                gc_sb = stp.tile([C, 1], F32)
                nc.scalar.activation(
                    out=gc_sb, in_=mt2[:C, 0:1], func=AF.Sigmoid, bias=b2_sb, scale=1.0
                )
                mt3 = psq.tile([128, MCH], F32, tag="ps")
                nc.tensor.matmul(
                    mt3[:, 0:1], lhsT=dup_sb, rhs=gc_sb, start=True, stop=True
                )
                g2_sb = stp.tile([128, 1], F32)
                nc.vector.tensor_copy(out=g2_sb, in_=mt3[:, 0:1])

                # ------- pass 2: apply both gates, stream out ---------------
                for jp in range(NPAIR):
                    r0 = 32 * jp
                    for pc in range(S // PCH):
                        o = pc * PCH
                        xc = xres[:, jp * S + o : jp * S + o + PCH]
                        g_ps = pb.tile([128, PCH], F32)
                        for i in range(PCH // 512):
                            nc.tensor.matmul(
                                g_ps[:, i * 512 : (i + 1) * 512],
                                lhsT=bsel_sb[r0 : r0 + 2, :],
                                rhs=sqb[r0 : r0 + 2, o + i * 512 : o + (i + 1) * 512],
                                start=True,
                                stop=True,
                                tile_position=(r0, 0),
                            )
                        t2 = t2p.tile([128, PCH], F32)
                        nc.vector.tensor_mul(out=t2, in0=xc, in1=g_ps)
                        nc.vector.scalar_tensor_tensor(
                            out=t2,
                            in0=xc,
                            scalar=g2_sb,
                            in1=t2,
                            op0=AL.mult,
                            op1=AL.max,
                        )
                        nc.sync.dma_start(
                            out=yout[jp, :, o : o + PCH],
                            in_=t2,
                        )
    # run Bacc's compile pipeline (register allocation, wait splitting);
    # the bass2jax/PJRT runner does not finalize on its own.
    nc.finalize()
    return nc


def _shard(x):
    # core k shard: xin[jp, 64*t + c, s] = x[b, c, d0 + 2*jp + t, s]
    in_maps = []
    for k in range(NCORES):
        b, d0 = k // 4, SL * (k % 4)
        v = x[b, :, d0 : d0 + SL].reshape(C, NPAIR, 2, S)
        shard = np.ascontiguousarray(v.transpose(1, 2, 0, 3).reshape(NPAIR, 128, S))
        in_maps.append({"xin": shard})
    return in_maps


def _unshard(results):
    out = np.empty((B, C, D, H, W), np.float32)
    for k in range(NCORES):
        b, d0 = k // 4, SL * (k % 4)
        y = results[k]["yout"].reshape(NPAIR, 2, C, S)
        out[b, :, d0 : d0 + SL] = y.transpose(2, 0, 1, 3).reshape(C, SL, H, W)
    return out


def _run(inputs, trace=False):
    x = np.ascontiguousarray(np.asarray(inputs["input_tensor"], dtype=np.float32))
    ws = [
        np.asarray(inputs[k], dtype=np.float32)
        for k in ("fc1_w", "fc1_b", "fc2_w", "fc2_b", "conv_w", "conv_b")
    ]
    nc = _build(*ws)
    res = run_bass_kernel_spmd(nc, _shard(x), list(range(NCORES)), trace=trace)
    return _unshard(res.results), res


def kernel(**inputs):
    out, _ = _run(inputs, trace=False)
    return out
